# revision 21
# baseline (speedup 1.0000x reference)
"""Box filter (radius 8, window 17, zero-padded edges) over dims 2,3 of a
[8, 32, 512, 512] f32 tensor, on 8 Trainium2 NeuronCores.

Decomposition (validated vs the jax reference):
  - Per-axis filter with clipped windows = multiplication by a banded ones
    matrix B (B[i,k] = 1 iff |i-k| <= 8), i.e. Z = B @ X @ B.
  - Column (free-dim) filter: DVE `tensor_tensor_scan` computes the sliding
    window sum via state[t] = (x[t] + state[t-1]) - x[t-17] over a buffer
    with 17 front zeros / 8 back zeros per 512-col block. Measured HW cost
    is 55 + 2.09*N ns (dtype-independent), so block pairs are chained into
    one wide scan each (the 17 front-pad zeros of the pair's second block
    drain the fp32 window state exactly across the boundary). The first and
    last channel use 4 per-block scans instead, shortening pipeline ramp
    and tail.
  - Row (partition-dim) filter: PE matmul. Block t holds input rows
    128t-8..128t+119 (top halo pre-shifted in), so one K=128 banded matmul
    covers an output tile except its last 16 rows; one K<=16 fix matmul
    accumulates those from the next block's partitions 0..15. Rows 504..511
    are stashed in block 0's partitions 0..7 and consumed by tile 3's fix
    (tile 0's main lhsT has rows 0..7 zeroed to ignore the stash).
  - Everything is bf16 end-to-end (input converted on host, output converted
    back): HBM traffic is halved to 16.8 MB in + 16.8 MB out per core, scan
    state stays fp32 inside the DVE, PSUM accumulates f32, and the Scalar
    engine downcasts PSUM->SBUF. Measured rel err ~4e-3 vs the 2e-2 gate.

Engine budget per channel (measured): DVE 2 scans ~4.5 us (critical path,
100%-busy: 144 us of the ~163 us total), PE 8 matmuls ~4.4 us, Scalar 4
copies ~2.8 us, DMA ~2.9 us across 16 queues. Output stores issue on the
otherwise-idle GpSimd queue (SWDGE path with its own descriptor rings: a
store can never queue behind prefetched loads, which once stalled the
Scalar queue 15 us via the og-buffer WAR chain); the last channel's stores
go back on Scalar/HWDGE since the load rings are empty by then. The input
load is split per block pair so scan A starts before blocks 2-3 arrive;
pad memsets run on GpSimd.

Sharding: data-parallel over batch (dim 0) -> 8 cores, one batch each.
"""

import os
import sys

import numpy as np

for _p in ("/opt/trn_rl_repo", "/root/.axon_site/_ro/trn_rl_repo"):
    if os.path.isdir(_p) and _p not in sys.path:
        sys.path.append(_p)

import ml_dtypes

import concourse.bass as bass
import concourse.tile as tile
from concourse import bacc, mybir
from concourse.bass_utils import run_bass_kernel_spmd

R = 8
PADF = 2 * R + 1  # front zero pad (window width)
PADB = R          # back zero pad
H = W = 512
CH = 32
NCORES = 8
XW = PADF + W + PADB  # 537
XALL = 4 * XW         # 2148
UW = XALL - PADF      # 2131: scan output width (u col XW*b+8+c = block b col c)
NBIG = 7
NU = 6
NOBIG = 6

_CACHE = {}


def _banded():
    # Block t partition k holds input row 128t - 8 + k. Main band:
    # |m - (k - 8)| <= 8  ->  k - 16 <= m <= k.
    k = np.arange(128)[:, None]
    m = np.arange(128)[None, :]
    bmain = ((m >= k - 2 * R) & (m <= k)).astype(np.float32)
    # Tile 0's main: partitions 0..7 hold the stashed rows 504..511, not
    # rows -8..-1 — zero them out (zero-pad semantics at the top edge).
    bmainf = bmain.copy()
    bmainf[0:R, :] = 0.0
    # Bottom fix for tile t<3: rhs partitions k (k=0..15) = input row
    # 128t + 120 + k; output rows 64+m (m=0..63, psum slice [64:128]):
    # window iff m + 64 >= k + 112  ->  m >= k + 48.
    # Both fix bands are K- and M-PADDED to [128, 128] with zero rows/cols:
    # matmul cost is N-only, and keeping every matmul the exact same shape
    # (K=128, M=128, N=512) avoids the PE stationary-array resize that
    # pinned matmuls at ~550 ns. Fix band: out row m gets stashed/halo row k
    # iff m >= k + 112 (m >= 112 implied). Padded rows accumulate exact 0.
    kb = np.arange(128)[:, None]
    mb = np.arange(128)[None, :]
    bbot = ((kb < 16) & (mb >= kb + 14 * R)).astype(np.float32)
    bbotl = ((kb < 8) & (mb >= kb + 14 * R)).astype(np.float32)
    bf = ml_dtypes.bfloat16
    return (bmainf.astype(bf), bmain.astype(bf),
            bbot.astype(bf), bbotl.astype(bf))


def _build_program():
    if "nc" in _CACHE:
        return _CACHE["nc"]
    nc = bacc.Bacc(debug=False)
    f32 = mybir.dt.float32
    bf16 = mybir.dt.bfloat16
    x = nc.dram_tensor("x", [CH, H, W], bf16, kind="ExternalInput")
    z = nc.dram_tensor("z", [CH, H, W], bf16, kind="ExternalOutput")
    bma = nc.dram_tensor("bma", [128, 128], bf16, kind="ExternalInput")
    bmb = nc.dram_tensor("bmb", [128, 128], bf16, kind="ExternalInput")
    bb2 = nc.dram_tensor("bb2", [128, 128], bf16, kind="ExternalInput")
    bbl = nc.dram_tensor("bbl", [128, 128], bf16, kind="ExternalInput")
    xap, zap = x.ap(), z.ap()

    with tile.TileContext(nc) as tc:
        with (
            tc.tile_pool(name="consts", bufs=1) as cpool,
            tc.tile_pool(name="psum", bufs=8, space="PSUM") as ppool,
        ):
            xalls = [
                nc.alloc_sbuf_tensor(f"xall{i}", [128, XALL], bf16).ap()
                for i in range(NBIG)
            ]
            bmat = cpool.tile([128, 128], bf16)
            bmbt = cpool.tile([128, 128], bf16)
            bb2t = cpool.tile([128, 128], bf16)
            bblt = cpool.tile([128, 128], bf16)
            # channel 0's scan-gating loads FIRST on the scalar queue:
            # their desc-gen runs concurrently with the sync queue's, so the
            # first scan starts ~1 us earlier. Consts follow (matmuls need
            # them only ~2 us later).
            nc.scalar.dma_start(
                xalls[0][8:128, PADF:PADF + W], xap[0, 0:120, :]
            )
            nc.scalar.dma_start(
                xalls[0][0:8, PADF:PADF + W], xap[0, 504:512, :]
            )
            nc.scalar.dma_start(bmat[:], bma.ap()[:, :])
            nc.scalar.dma_start(bmbt[:], bmb.ap()[:, :])
            nc.scalar.dma_start(bb2t[:], bb2.ap()[:, :])
            nc.scalar.dma_start(bblt[:], bbl.ap()[:, :])

            # Static ring. Loads only ever touch the data columns, so the
            # 17+8 zero pads around each 512-col block are zeroed ONCE here
            # (on GpSimd: keeps the DVE queue clear).

            for xa in xalls:
                nc.gpsimd.memset(xa[:, 0:PADF], 0.0)
                mid = bass.AP(
                    tensor=xa.tensor,
                    offset=xa.offset + XW - PADB,
                    ap=[[XALL, 128], [XW, 3], [1, PADF + PADB]],
                )
                nc.gpsimd.memset(mid, 0.0)
                nc.gpsimd.memset(xa[:, XALL - PADB:XALL], 0.0)
            us = [
                nc.alloc_sbuf_tensor(f"u{i}", [128, UW], bf16).ap()
                for i in range(NU)
            ]
            obigs = [
                nc.alloc_sbuf_tensor(f"obig{i}", [128, 4, W], bf16).ap()
                for i in range(NOBIG)
            ]

            PAIRW = 2 * XW - PADF  # 1057: width of one block-pair scan

            def scan_part(u, xa, off, width):
                # out col (off+t) = window ending at xa[off+PADF+t]; valid
                # whenever xa[off..off+PADF-1] lie in a zero pad region.
                nc.vector.tensor_tensor_scan(
                    out=u[0:128, off:off + width],
                    data0=xa[0:128, off + PADF:off + PADF + width],
                    data1=xa[0:128, off:off + width],
                    initial=0.0,
                    op0=mybir.AluOpType.add,
                    op1=mybir.AluOpType.subtract,
                )

            for c in range(CH):
                xa = xalls[c % NBIG]
                u = us[c % NU]
                og = obigs[c % NOBIG]

                # Four loads per channel (shifted-block layout):
                #   rows 0..119   -> block 0 partitions 8..127
                #   rows 504..511 -> block 0 partitions 0..7 (stash)
                #   rows 120..247 -> block 1 (gates scan A with block 0)
                #   rows 248..503 -> blocks 2..3 (gates scan B)
                if c > 0:  # c == 0's block-0 + stash loads issued at the top
                    nc.sync.dma_start(
                        xa[8:128, PADF:PADF + W], xap[c, 0:120, :]
                    )
                    nc.sync.dma_start(
                        xa[0:8, PADF:PADF + W], xap[c, 504:512, :]
                    )
                nc.sync.dma_start(
                    xa[:, XW + PADF:XW + PADF + W], xap[c, 120:248, :]
                )
                src = bass.AP(
                    tensor=x,
                    offset=(c * H + 248) * W,
                    ap=[[W, 128], [128 * W, 2], [1, W]],
                )
                dst = bass.AP(
                    tensor=xa.tensor,
                    offset=xa.offset + 2 * XW + PADF,
                    ap=[[XALL, 128], [XW, 2], [1, W]],
                )
                nc.sync.dma_start(dst, src)

                # Column scans. Middle channels: one wide scan per block
                # pair; first/last channel: per-block scans (finer deps ->
                # shorter pipeline ramp and tail).
                if c in (0, CH - 1):
                    for b in range(4):
                        scan_part(u, xa, b * XW, XW - PADF)
                else:
                    scan_part(u, xa, 0, PAIRW)
                    scan_part(u, xa, 2 * XW, PAIRW)

                for t in range(4):
                    ps = ppool.tile([128, W], f32)
                    nc.tensor.matmul(
                        ps[0:128, :],
                        (bmat if t == 0 else bmbt)[0:128, 0:128],
                        u[0:128, XW * t + R:XW * t + R + W],
                        start=True, stop=False, skip_group_check=True,
                    )
                    if t < 3:
                        nc.tensor.matmul(
                            ps[0:128, :], bb2t[0:128, 0:128],
                            u[0:128, XW * (t + 1) + R:XW * (t + 1) + R + W],
                            start=False, stop=True, skip_group_check=True,
                        )
                    else:
                        nc.tensor.matmul(
                            ps[0:128, :], bblt[0:128, 0:128],
                            u[0:128, R:R + W],
                            start=False, stop=True, skip_group_check=True,
                        )
                    nc.scalar.copy(og[:, t, :], ps[0:128, :])
                    if c == CH - 1:
                        # last channel: per-tile stores so the kernel tail
                        # ends on a small transfer. Sync queue: empty at the
                        # tail, so the 0.67us desc-gens don't delay the
                        # remaining copies on the scalar queue.
                        nc.sync.dma_start(
                            zap[c, 128 * t:128 * t + 128, :], og[:, t, :]
                        )

                if c < CH - 1:
                    # ONE batched store per channel on the otherwise-idle
                    # GpSimd queue: SWDGE descriptors use their own ring, so
                    # a store can never block behind prefetched loads (which
                    # stalled the Scalar queue 15 us when stores shared its
                    # path), and the Scalar sequencer keeps only the copies.
                    nc.gpsimd.dma_start(
                        zap[c, :, :].rearrange("(t p) w -> p t w", p=128),
                        og[:, :, :],
                    )

    nc.compile()
    _CACHE["nc"] = nc
    return nc


def kernel(tensor: np.ndarray) -> np.ndarray:
    tensor = np.asarray(tensor)
    assert tensor.shape == (NCORES, CH, H, W)
    xb = tensor.astype(ml_dtypes.bfloat16)
    bmaf, bmab, bb2, bbl = _banded()
    nc = _build_program()
    in_maps = [
        {"x": xb[i], "bma": bmaf, "bmb": bmab, "bb2": bb2, "bbl": bbl}
        for i in range(NCORES)
    ]
    res = run_bass_kernel_spmd(nc, in_maps, core_ids=list(range(NCORES)))
    return np.stack(
        [res.results[i]["z"].astype(np.float32) for i in range(NCORES)], axis=0
    )
